# revision 17
# baseline (speedup 1.0000x reference)
"""BitLinear (int8-activation x ternary-weight) matmul on 8 TRN2 NeuronCores.

Full inputs: x [4, 4096, 2048] f32, weight [2048, 2048] f32.
Output: [4, 4096, 2048] fp16 = ((qx @ qw.T) / si / sw).astype(f16).

Data-parallel over the 16384 rows (2048 rows/core). The weight is
replicated; each core computes mean|W| on-device during a single
streaming W read, ternarizes W into fp8 {-1,0,1}, and runs
bf16(lhsT=qx^T) x fp8(qw^T) matmuls with fp32 PSUM accumulation
(exact for these integer values). Dequant (acc * amax/127 * mean|W|)
is fused into the PSUM->SBUF fp16 copy on the ScalarEngine.

Head scheduling (what decides time-to-first-matmul, measured on HW):
- ALL input loads ride the sync HWDGE ring alone, in order
  W11..W15, x0, W0..W10. The five tiles that cannot stay cached in
  SBUF (11..15) are loaded FIRST through a 3-buf bounce pool: their
  |w|-reduces retire early, so no load ever waits on a DVE reduce of
  a previous bounce occupant (the 2-buf tail rotation cost v3 ~13us
  of serial DMA<->DVE interlock), and the re-reads after sw recycle
  the same bufs without Sign-gating stalls.
- DMA-xbar transposes are emitted after the W stream (the transpose
  serialization guard makes later HWDGE DMAs wait on them), and the
  x2 load follows the re-reads immediately so row 2 is never input-
  blocked when the ramp retires.
- Putting W loads on the scalar ring was measured WORSE (ACT-queue
  head-of-line + semaphore-lane aliasing with the transpose guard).
- Rows 0,1 interleave across arriving ternarized k-tiles with row 1
  lagging 4 tiles (its qxT dependency never head-of-line blocks row
  0's stream in the PE queue); rows 2..15 then run back-to-back at
  the PE rate (~216ns per [128x128]x[128x512] bf16 matmul).

Activation quantization uses the fp32 magic-number trick on DVE
(x*si + 1.5*2^23 rounds to nearest-even) with the un-bias into bf16
on ACT. fp8 DoubleRow was tried: it trips the board GPIO power
throttle (PE clamped to 13/16 clock), cancelling the gain.
"""

import numpy as np

import concourse.mybir as mybir
import concourse.tile as tile
from concourse import bacc
from concourse.bass import ts
from concourse.bass_utils import run_bass_kernel_spmd

N_CORES = 8
ROWS_TOTAL = 4 * 4096
K = 2048
N = 2048
MAGIC = 12582912.0  # 1.5*2^23: fp32 round-to-nearest-even (both signs)

KT = K // 128  # 16 k-tiles
NQ = N // 512  # 4 psum-width output chunks
WCACHE = 11  # raw W k-tiles 0..WCACHE-1 held in SBUF; the rest re-read
LAG = 4  # row-1 k-tile lag in the ramp

f32 = mybir.dt.float32
bf16 = mybir.dt.bfloat16
f16 = mybir.dt.float16
fp8 = mybir.dt.float8e4
Alu = mybir.AluOpType
Act = mybir.ActivationFunctionType
AxX = mybir.AxisListType.X


def build(rows_per_core=ROWS_TOTAL // N_CORES):
    nc = bacc.Bacc(
        "TRN2", target_bir_lowering=False, debug=False, num_devices=N_CORES
    )
    x_ext = nc.declare_dram_parameter("x", [rows_per_core, K], f32, isOutput=False)
    wt_ext = nc.declare_dram_parameter("wt", [K, N], f32, isOutput=False)
    out_ext = nc.declare_dram_parameter(
        "out", [rows_per_core, N], f16, isOutput=True
    )

    MT = rows_per_core // 128

    with tile.TileContext(nc) as tc:
        with (
            tc.tile_pool(name="xin", bufs=2) as xin,  # [128,K] f32 x loads
            tc.tile_pool(name="wld", bufs=WCACHE) as wld,  # cached raw W
            tc.tile_pool(name="wtmp", bufs=5) as wtmp,  # W 11..15 bounce
            tc.tile_pool(name="scaled", bufs=1) as scaled,  # x*si+MAGIC f32
            tc.tile_pool(name="qtmp", bufs=1) as qtmp,  # qx bf16
            tc.tile_pool(name="qxt", bufs=3) as qxtp,  # [128,KT,128] bf16 x^T
            tc.tile_pool(name="outp", bufs=1) as outp,  # [128,N] f16 results
            tc.tile_pool(name="singles", bufs=1) as singles,
            tc.tile_pool(name="small", bufs=8) as small,  # [128,1] stats
            tc.tile_pool(name="pacc", bufs=8, space="PSUM") as pacc,
        ):
            ones_mat = singles.tile([128, 128], f32)
            nc.vector.memset(ones_mat, 1.0)
            negmagic_b = singles.tile([128, 1], f32)
            nc.vector.memset(negmagic_b, -MAGIC)
            wsums = singles.tile([128, KT], f32)
            qwT = singles.tile([128, KT, N], fp8)

            w_tiles = {}

            def w_load(kt, pool, tag):
                wt_t = pool.tile([128, K], f32, tag=tag, name=f"w{kt}")
                nc.sync.dma_start(out=wt_t, in_=wt_ext[ts(kt, 128), :])
                w_tiles[kt] = wt_t

            x_tiles = {}

            def x_load(mi):
                x_t = xin.tile([128, K], f32, tag="xin", name=f"x{mi}")
                nc.sync.dma_start(out=x_t, in_=x_ext[ts(mi, 128), :])
                x_tiles[mi] = x_t

            # ---- sync ring: uncached W tiles first, x0 early, then the
            # cached W tiles
            w_load(WCACHE, wtmp, "wtmp")
            w_load(WCACHE + 1, wtmp, "wtmp")
            x_load(0)
            for kt in range(WCACHE + 2, KT):
                w_load(kt, wtmp, "wtmp")
            for kt in range(WCACHE):
                w_load(kt, wld, "wld")

            # ---- x-quant pieces
            amcs = {}

            def xq_dve(mi):
                x_t = x_tiles[mi]
                amax = small.tile([128, 1], f32, tag="small")
                nc.vector.tensor_reduce(
                    out=amax, in_=x_t, axis=AxX, op=Alu.max,
                    apply_absolute_value=True,
                )
                amc = small.tile([128, 1], f32, tag="amc", name=f"amc{mi}")
                nc.vector.tensor_scalar_max(out=amc, in0=amax, scalar1=1e-5)
                rec = small.tile([128, 1], f32, tag="small")
                nc.vector.reciprocal(out=rec, in_=amc)
                si = small.tile([128, 1], f32, tag="small")
                nc.vector.tensor_scalar_mul(out=si, in0=rec, scalar1=127.0)
                xs = scaled.tile([128, K], f32, tag="scaled")
                nc.vector.tensor_scalar(
                    out=xs, in0=x_t, scalar1=si, scalar2=MAGIC,
                    op0=Alu.mult, op1=Alu.add,
                )
                amcs[mi] = amc
                return xs

            qxs = {}

            def xq_act(mi, xs):
                qx = qtmp.tile([128, K], bf16, tag="qtmp")
                nc.scalar.activation(out=qx, in_=xs, func=Act.Copy, bias=-MAGIC)
                qxs[mi] = qx

            qxTs = {}

            def xq_transpose(mi):
                qxT = qxtp.tile([128, KT, 128], bf16, tag="qxt", name=f"qxT{mi}")
                nc.sync.dma_start_transpose(out=qxT, in_=qxs.pop(mi))
                qxTs[mi] = qxT

            # ---- mean|W| during the stream: bounce tiles first (their
            # reduces must retire early to keep the bounce rotation off the
            # load critical path), x0's quant chain next (it fills the DVE
            # gap before the cached tiles arrive), then the cached tiles
            def ws_reduce(kt):
                nc.vector.tensor_reduce(
                    out=wsums[:, kt : kt + 1], in_=w_tiles[kt], axis=AxX,
                    op=Alu.add, apply_absolute_value=True,
                )

            for kt in range(WCACHE, KT):
                ws_reduce(kt)
            xq_act(0, xq_dve(0))
            for kt in range(WCACHE):
                ws_reduce(kt)
            wtot = small.tile([128, 1], f32, tag="small")
            nc.vector.tensor_reduce(out=wtot, in_=wsums, axis=AxX, op=Alu.add)
            # ones_mat.T @ wtot replicates the grand total across all 128
            # partitions so the scale math stays [128,1]
            ptot_b = pacc.tile([128, 1], f32, tag="acc", name="ptot_b")
            nc.tensor.matmul(ptot_b, lhsT=ones_mat, rhs=wtot, start=True, stop=True)
            meanc_b = small.tile([128, 1], f32, tag="s1")
            nc.vector.tensor_scalar(
                out=meanc_b, in0=ptot_b, scalar1=1.0 / (K * N), scalar2=1e-5,
                op0=Alu.mult, op1=Alu.max,
            )
            sw_b = singles.tile([128, 1], f32)
            nc.vector.reciprocal(out=sw_b, in_=meanc_b)
            q_b = singles.tile([128, 1], f32)
            nc.vector.tensor_scalar_mul(out=q_b, in0=meanc_b, scalar1=1.0 / 127.0)

            # ---- W pass2: u = w*sw + MAGIC (DVE in-place), then
            # Sign(u - MAGIC) -> fp8 (ACT); for integer n, sign(n) ==
            # clip(n, -1, 1)
            def wq(kt):
                wt_t = w_tiles[kt]
                nc.vector.tensor_scalar(
                    out=wt_t, in0=wt_t, scalar1=sw_b, scalar2=MAGIC,
                    op0=Alu.mult, op1=Alu.add,
                )
                nc.scalar.activation(
                    out=qwT[:, kt, :], in_=wt_t, func=Act.Sign, bias=negmagic_b
                )

            # sync ring after the W stream: x0's transpose (gates the first
            # matmul), x1 + transpose, the W11..15 re-reads, then x2 etc.
            xq_transpose(0)
            x_load(1)
            wq(0)
            wq(1)
            xq_act(1, xq_dve(1))
            xq_transpose(1)
            for kt in range(WCACHE, KT):
                w_load(kt, wtmp, "wtmp")  # re-read for ternarization
            for kt in range(2, KT):
                wq(kt)

            # ---- matmuls
            def mm(acc, qxT, kt, nq):
                nc.tensor.matmul(
                    acc, lhsT=qxT[:, kt, :], rhs=qwT[:, kt, ts(nq, 512)],
                    start=(kt == 0), stop=(kt == KT - 1),
                    skip_group_check=True,
                )

            def mk_accs(mi):
                return [
                    pacc.tile([128, 512], f32, tag="acc", name=f"acc_{mi}_{i}")
                    for i in range(NQ)
                ]

            def finish(mi, accs, chunked=False):
                cs = small.tile([128, 1], f32, tag="small")
                nc.vector.tensor_mul(cs, amcs.pop(mi), q_b)  # (amax/127)*meanc
                o_t = outp.tile([128, N], f16, tag="outp", name=f"o{mi}")
                for nq in range(NQ):
                    nc.scalar.activation(
                        out=o_t[:, ts(nq, 512)], in_=accs[nq],
                        func=Act.Copy, scale=cs,
                    )
                    if chunked:
                        nc.scalar.dma_start(
                            out=out_ext[ts(mi, 128), ts(nq, 512)],
                            in_=o_t[:, ts(nq, 512)],
                        )
                if not chunked:
                    nc.scalar.dma_start(out=out_ext[ts(mi, 128), :], in_=o_t)

            # ramp: rows 0,1 interleaved per k-tile, row 1 lagging LAG tiles
            accs0 = mk_accs(0)
            accs1 = mk_accs(1)
            for step in range(KT + LAG):
                if step < KT:
                    for nq in range(NQ):
                        mm(accs0[nq], qxTs[0], step, nq)
                if step >= LAG:
                    for nq in range(NQ):
                        mm(accs1[nq], qxTs[1], step - LAG, nq)
            finish(0, accs0)
            finish(1, accs1)
            del qxTs[0], qxTs[1]

            # steady rows
            for mi in range(2, MT):
                x_load(mi)
                xq_act(mi, xq_dve(mi))
                xq_transpose(mi)
                qxT = qxTs.pop(mi)
                accs = mk_accs(mi)
                if mi == MT - 1:
                    # nq-inner: each output chunk completes as soon as its
                    # 16 accumulations are done (shorter kernel tail)
                    for nq in range(NQ):
                        for kt in range(KT):
                            mm(accs[nq], qxT, kt, nq)
                else:
                    for kt in range(KT):
                        for nq in range(NQ):
                            mm(accs[nq], qxT, kt, nq)
                finish(mi, accs, chunked=(mi == MT - 1))

    nc.compile()
    return nc


_NC_CACHE = {}


def _get_nc(rows_per_core):
    if rows_per_core not in _NC_CACHE:
        _NC_CACHE[rows_per_core] = build(rows_per_core)
    return _NC_CACHE[rows_per_core]


def run(x, weight, **spmd_kwargs):
    x = np.ascontiguousarray(np.asarray(x, dtype=np.float32))
    weight = np.asarray(weight, dtype=np.float32)
    b, s, k = x.shape
    rows = b * s
    rpc = rows // N_CORES
    xr = x.reshape(rows, k)
    wt = np.ascontiguousarray(weight.T)
    nc = _get_nc(rpc)
    in_maps = [
        {"x": xr[i * rpc : (i + 1) * rpc], "wt": wt} for i in range(N_CORES)
    ]
    res = run_bass_kernel_spmd(
        nc, in_maps, core_ids=list(range(N_CORES)), **spmd_kwargs
    )
    out = np.concatenate(
        [res.results[i]["out"] for i in range(N_CORES)], axis=0
    )
    return out.reshape(b, s, N), res


def kernel(x, weight):
    out, _ = run(x, weight)
    return out


# revision 19
# speedup vs baseline: 1.0124x; 1.0124x over previous
"""BitLinear (int8-activation x ternary-weight) matmul on 8 TRN2 NeuronCores.

Full inputs: x [4, 4096, 2048] f32, weight [2048, 2048] f32.
Output: [4, 4096, 2048] fp16 = ((qx @ qw.T) / si / sw).astype(f16).

Data-parallel over the 16384 rows (2048 rows/core). The weight is
replicated; each core computes mean|W| on-device during a single
streaming W read, ternarizes W into fp8 {-1,0,1}, and runs
bf16(lhsT=qx^T) x fp8(qw^T) matmuls with fp32 PSUM accumulation
(exact for these integer values). Dequant (acc * amax/127 * mean|W|)
is fused into the PSUM->SBUF fp16 copy on the ScalarEngine.

Head scheduling (what decides time-to-first-matmul, measured on HW):
- ALL input loads ride the sync HWDGE ring alone, in order
  W11..W15, x0, W0..W10. The five tiles that cannot stay cached in
  SBUF (11..15) are loaded FIRST through a 3-buf bounce pool: their
  |w|-reduces retire early, so no load ever waits on a DVE reduce of
  a previous bounce occupant (the 2-buf tail rotation cost v3 ~13us
  of serial DMA<->DVE interlock), and the re-reads after sw recycle
  the same bufs without Sign-gating stalls.
- DMA-xbar transposes are emitted after the W stream (the transpose
  serialization guard makes later HWDGE DMAs wait on them), and the
  x2 load follows the re-reads immediately so row 2 is never input-
  blocked when the ramp retires.
- Putting W loads on the scalar ring was measured WORSE (ACT-queue
  head-of-line + semaphore-lane aliasing with the transpose guard).
- Rows 0,1 interleave across arriving ternarized k-tiles with row 1
  lagging 4 tiles (its qxT dependency never head-of-line blocks row
  0's stream in the PE queue); rows 2..15 then run back-to-back at
  the PE rate (~216ns per [128x128]x[128x512] bf16 matmul).

Activation quantization uses the fp32 magic-number trick on DVE
(x*si + 1.5*2^23 rounds to nearest-even) with the un-bias into bf16
on ACT. fp8 DoubleRow was tried: it trips the board GPIO power
throttle (PE clamped to 13/16 clock), cancelling the gain.
"""

import numpy as np

import concourse.mybir as mybir
import concourse.tile as tile
from concourse import bacc
from concourse.bass import ts
from concourse.bass_utils import run_bass_kernel_spmd

N_CORES = 8
ROWS_TOTAL = 4 * 4096
K = 2048
N = 2048
MAGIC = 12582912.0  # 1.5*2^23: fp32 round-to-nearest-even (both signs)

KT = K // 128  # 16 k-tiles
NQ = N // 512  # 4 psum-width output chunks
WCACHE = 11  # raw W k-tiles 0..WCACHE-1 held in SBUF; the rest re-read
LAG = 4  # row-1 k-tile lag in the ramp

f32 = mybir.dt.float32
bf16 = mybir.dt.bfloat16
f16 = mybir.dt.float16
fp8 = mybir.dt.float8e4
Alu = mybir.AluOpType
Act = mybir.ActivationFunctionType
AxX = mybir.AxisListType.X


def build(rows_per_core=ROWS_TOTAL // N_CORES):
    nc = bacc.Bacc(
        "TRN2", target_bir_lowering=False, debug=False, num_devices=N_CORES
    )
    x_ext = nc.declare_dram_parameter("x", [rows_per_core, K], f32, isOutput=False)
    wt_ext = nc.declare_dram_parameter("wt", [K, N], f32, isOutput=False)
    out_ext = nc.declare_dram_parameter(
        "out", [rows_per_core, N], f16, isOutput=True
    )

    MT = rows_per_core // 128

    with tile.TileContext(nc) as tc:
        with (
            tc.tile_pool(name="xin", bufs=3) as xin,  # [128,K] f32 x loads
            tc.tile_pool(name="wld", bufs=WCACHE) as wld,  # cached raw W
            tc.tile_pool(name="wtmp", bufs=4) as wtmp,  # W 11..15 bounce
            tc.tile_pool(name="scaled", bufs=1) as scaled,  # x*si+MAGIC f32
            tc.tile_pool(name="qtmp", bufs=1) as qtmp,  # qx bf16
            tc.tile_pool(name="qxt", bufs=3) as qxtp,  # [128,KT,128] bf16 x^T
            tc.tile_pool(name="outp", bufs=1) as outp,  # [128,N] f16 results
            tc.tile_pool(name="singles", bufs=1) as singles,
            tc.tile_pool(name="small", bufs=8) as small,  # [128,1] stats
            tc.tile_pool(name="pacc", bufs=8, space="PSUM") as pacc,
        ):
            ones_mat = singles.tile([128, 128], f32)
            nc.vector.memset(ones_mat, 1.0)
            negmagic_b = singles.tile([128, 1], f32)
            nc.vector.memset(negmagic_b, -MAGIC)
            wsums = singles.tile([128, KT], f32)
            qwT = singles.tile([128, KT, N], fp8)

            w_tiles = {}

            def w_load(kt, pool, tag):
                wt_t = pool.tile([128, K], f32, tag=tag, name=f"w{kt}")
                nc.sync.dma_start(out=wt_t, in_=wt_ext[ts(kt, 128), :])
                w_tiles[kt] = wt_t

            x_tiles = {}

            def x_load(mi):
                x_t = xin.tile([128, K], f32, tag="xin", name=f"x{mi}")
                nc.sync.dma_start(out=x_t, in_=x_ext[ts(mi, 128), :])
                x_tiles[mi] = x_t

            # ---- sync ring: uncached W tiles first, x0 early, then the
            # cached W tiles
            w_load(WCACHE, wtmp, "wtmp")
            w_load(WCACHE + 1, wtmp, "wtmp")
            x_load(0)
            for kt in range(WCACHE + 2, KT):
                w_load(kt, wtmp, "wtmp")
            for kt in range(WCACHE):
                w_load(kt, wld, "wld")

            # ---- x-quant pieces
            amcs = {}

            def xq_dve(mi):
                x_t = x_tiles[mi]
                amax = small.tile([128, 1], f32, tag="small")
                nc.vector.tensor_reduce(
                    out=amax, in_=x_t, axis=AxX, op=Alu.max,
                    apply_absolute_value=True,
                )
                amc = small.tile([128, 1], f32, tag="amc", name=f"amc{mi}")
                nc.vector.tensor_scalar_max(out=amc, in0=amax, scalar1=1e-5)
                rec = small.tile([128, 1], f32, tag="small")
                nc.vector.reciprocal(out=rec, in_=amc)
                si = small.tile([128, 1], f32, tag="small")
                nc.vector.tensor_scalar_mul(out=si, in0=rec, scalar1=127.0)
                xs = scaled.tile([128, K], f32, tag="scaled")
                nc.vector.tensor_scalar(
                    out=xs, in0=x_t, scalar1=si, scalar2=MAGIC,
                    op0=Alu.mult, op1=Alu.add,
                )
                amcs[mi] = amc
                return xs

            qxs = {}

            def xq_act(mi, xs):
                qx = qtmp.tile([128, K], bf16, tag="qtmp")
                nc.scalar.activation(out=qx, in_=xs, func=Act.Copy, bias=-MAGIC)
                qxs[mi] = qx

            qxTs = {}

            def xq_transpose(mi):
                qxT = qxtp.tile([128, KT, 128], bf16, tag="qxt", name=f"qxT{mi}")
                nc.sync.dma_start_transpose(out=qxT, in_=qxs.pop(mi))
                qxTs[mi] = qxT

            # ---- mean|W| during the stream: bounce tiles first (their
            # reduces must retire early to keep the bounce rotation off the
            # load critical path), x0's quant chain next (it fills the DVE
            # gap before the cached tiles arrive), then the cached tiles
            def ws_reduce(kt):
                nc.vector.tensor_reduce(
                    out=wsums[:, kt : kt + 1], in_=w_tiles[kt], axis=AxX,
                    op=Alu.add, apply_absolute_value=True,
                )

            for kt in range(WCACHE, KT):
                ws_reduce(kt)
            xq_act(0, xq_dve(0))
            for kt in range(WCACHE):
                ws_reduce(kt)
            wtot = small.tile([128, 1], f32, tag="small")
            nc.vector.tensor_reduce(out=wtot, in_=wsums, axis=AxX, op=Alu.add)
            # ones_mat.T @ wtot replicates the grand total across all 128
            # partitions so the scale math stays [128,1]
            ptot_b = pacc.tile([128, 1], f32, tag="acc", name="ptot_b")
            nc.tensor.matmul(ptot_b, lhsT=ones_mat, rhs=wtot, start=True, stop=True)
            meanc_b = small.tile([128, 1], f32, tag="s1")
            nc.vector.tensor_scalar(
                out=meanc_b, in0=ptot_b, scalar1=1.0 / (K * N), scalar2=1e-5,
                op0=Alu.mult, op1=Alu.max,
            )
            sw_b = singles.tile([128, 1], f32)
            nc.vector.reciprocal(out=sw_b, in_=meanc_b)
            q_b = singles.tile([128, 1], f32)
            nc.vector.tensor_scalar_mul(out=q_b, in0=meanc_b, scalar1=1.0 / 127.0)

            # ---- W pass2: u = w*sw + MAGIC (DVE in-place), then
            # Sign(u - MAGIC) -> fp8 (ACT); for integer n, sign(n) ==
            # clip(n, -1, 1)
            def wq(kt):
                # u = w*sw + MAGIC split across DVE and the otherwise-idle
                # GPSIMD (0.42-efficiency Q7 path balances 1536/512 cols):
                # the serial Sign-unlock chain is what paces the ramp
                wt_t = w_tiles[kt]
                nc.vector.tensor_scalar(
                    out=wt_t[:, :1536], in0=wt_t[:, :1536], scalar1=sw_b,
                    scalar2=MAGIC, op0=Alu.mult, op1=Alu.add,
                )
                nc.gpsimd.tensor_scalar(
                    out=wt_t[:, 1536:], in0=wt_t[:, 1536:], scalar1=sw_b,
                    scalar2=MAGIC, op0=Alu.mult, op1=Alu.add,
                )
                nc.scalar.activation(
                    out=qwT[:, kt, :], in_=wt_t, func=Act.Sign, bias=negmagic_b
                )

            # sync ring after the W stream: x0's transpose (gates the first
            # matmul), x1 + transpose, the W11..15 re-reads, then x2 etc.
            xq_transpose(0)
            x_load(1)
            wq(0)
            wq(1)
            xq_act(1, xq_dve(1))
            xq_transpose(1)
            for kt in range(WCACHE, KT):
                w_load(kt, wtmp, "wtmp")  # re-read for ternarization
            for kt in range(2, KT):
                wq(kt)

            # ---- matmuls
            def mm(acc, qxT, kt, nq):
                nc.tensor.matmul(
                    acc, lhsT=qxT[:, kt, :], rhs=qwT[:, kt, ts(nq, 512)],
                    start=(kt == 0), stop=(kt == KT - 1),
                    skip_group_check=True,
                )

            def mk_accs(mi):
                return [
                    pacc.tile([128, 512], f32, tag="acc", name=f"acc_{mi}_{i}")
                    for i in range(NQ)
                ]

            def finish(mi, accs, chunked=False):
                cs = small.tile([128, 1], f32, tag="small")
                nc.vector.tensor_mul(cs, amcs.pop(mi), q_b)  # (amax/127)*meanc
                o_t = outp.tile([128, N], f16, tag="outp", name=f"o{mi}")
                for nq in range(NQ):
                    nc.scalar.activation(
                        out=o_t[:, ts(nq, 512)], in_=accs[nq],
                        func=Act.Copy, scale=cs,
                    )
                    if chunked:
                        nc.scalar.dma_start(
                            out=out_ext[ts(mi, 128), ts(nq, 512)],
                            in_=o_t[:, ts(nq, 512)],
                        )
                if not chunked:
                    nc.scalar.dma_start(out=out_ext[ts(mi, 128), :], in_=o_t)

            # ramp: rows 0,1 interleaved per k-tile, row 1 lagging LAG tiles
            accs0 = mk_accs(0)
            accs1 = mk_accs(1)
            for step in range(KT + LAG):
                if step < KT:
                    for nq in range(NQ):
                        mm(accs0[nq], qxTs[0], step, nq)
                if step >= LAG:
                    for nq in range(NQ):
                        mm(accs1[nq], qxTs[1], step - LAG, nq)
            finish(0, accs0)
            finish(1, accs1)
            del qxTs[0], qxTs[1]

            # steady rows
            for mi in range(2, MT):
                x_load(mi)
                xq_act(mi, xq_dve(mi))
                xq_transpose(mi)
                qxT = qxTs.pop(mi)
                accs = mk_accs(mi)
                if mi == MT - 1:
                    # nq-inner: each output chunk completes as soon as its
                    # 16 accumulations are done (shorter kernel tail)
                    for nq in range(NQ):
                        for kt in range(KT):
                            mm(accs[nq], qxT, kt, nq)
                else:
                    for kt in range(KT):
                        for nq in range(NQ):
                            mm(accs[nq], qxT, kt, nq)
                finish(mi, accs, chunked=(mi == MT - 1))

    nc.compile()
    return nc


_NC_CACHE = {}


def _get_nc(rows_per_core):
    if rows_per_core not in _NC_CACHE:
        _NC_CACHE[rows_per_core] = build(rows_per_core)
    return _NC_CACHE[rows_per_core]


def run(x, weight, **spmd_kwargs):
    x = np.ascontiguousarray(np.asarray(x, dtype=np.float32))
    weight = np.asarray(weight, dtype=np.float32)
    b, s, k = x.shape
    rows = b * s
    rpc = rows // N_CORES
    xr = x.reshape(rows, k)
    wt = np.ascontiguousarray(weight.T)
    nc = _get_nc(rpc)
    in_maps = [
        {"x": xr[i * rpc : (i + 1) * rpc], "wt": wt} for i in range(N_CORES)
    ]
    res = run_bass_kernel_spmd(
        nc, in_maps, core_ids=list(range(N_CORES)), **spmd_kwargs
    )
    out = np.concatenate(
        [res.results[i]["out"] for i in range(N_CORES)], axis=0
    )
    return out.reshape(b, s, N), res


def kernel(x, weight):
    out, _ = run(x, weight)
    return out


# revision 21
# speedup vs baseline: 1.0127x; 1.0003x over previous
"""BitLinear (int8-activation x ternary-weight) matmul on 8 TRN2 NeuronCores.

Full inputs: x [4, 4096, 2048] f32, weight [2048, 2048] f32.
Output: [4, 4096, 2048] fp16 = ((qx @ qw.T) / si / sw).astype(f16).

Data-parallel over the 16384 rows (2048 rows/core). The weight is
replicated; each core computes mean|W| on-device during a single
streaming W read, ternarizes W into fp8 {-1,0,1}, and runs
bf16(lhsT=qx^T) x fp8(qw^T) matmuls with fp32 PSUM accumulation
(exact for these integer values). Dequant (acc * amax/127 * mean|W|)
is fused into the PSUM->SBUF fp16 copy on the ScalarEngine.

Head scheduling (what decides time-to-first-matmul, measured on HW):
- ALL input loads ride the sync HWDGE ring alone, in order
  W11..W15, x0, W0..W10. The five tiles that cannot stay cached in
  SBUF (11..15) are loaded FIRST through a 3-buf bounce pool: their
  |w|-reduces retire early, so no load ever waits on a DVE reduce of
  a previous bounce occupant (the 2-buf tail rotation cost v3 ~13us
  of serial DMA<->DVE interlock), and the re-reads after sw recycle
  the same bufs without Sign-gating stalls.
- DMA-xbar transposes are emitted after the W stream (the transpose
  serialization guard makes later HWDGE DMAs wait on them), and the
  x2 load follows the re-reads immediately so row 2 is never input-
  blocked when the ramp retires.
- Putting W loads on the scalar ring was measured WORSE (ACT-queue
  head-of-line + semaphore-lane aliasing with the transpose guard).
- Rows 0,1 interleave across arriving ternarized k-tiles with row 1
  lagging 4 tiles (its qxT dependency never head-of-line blocks row
  0's stream in the PE queue); rows 2..15 then run back-to-back at
  the PE rate (~216ns per [128x128]x[128x512] bf16 matmul).

Activation quantization uses the fp32 magic-number trick on DVE
(x*si + 1.5*2^23 rounds to nearest-even) with the un-bias into bf16
on ACT. fp8 DoubleRow was tried: it trips the board GPIO power
throttle (PE clamped to 13/16 clock), cancelling the gain.
"""

import numpy as np

import concourse.mybir as mybir
import concourse.tile as tile
from concourse import bacc
from concourse import bass_isa
from concourse.bass import ts
from concourse.bass_utils import run_bass_kernel_spmd

N_CORES = 8
ROWS_TOTAL = 4 * 4096
K = 2048
N = 2048
MAGIC = 12582912.0  # 1.5*2^23: fp32 round-to-nearest-even (both signs)

KT = K // 128  # 16 k-tiles
NQ = N // 512  # 4 psum-width output chunks
WCACHE = 11  # raw W k-tiles 0..WCACHE-1 held in SBUF; the rest re-read
LAG = 4  # row-1 k-tile lag in the ramp

f32 = mybir.dt.float32
bf16 = mybir.dt.bfloat16
f16 = mybir.dt.float16
fp8 = mybir.dt.float8e4
Alu = mybir.AluOpType
Act = mybir.ActivationFunctionType
AxX = mybir.AxisListType.X


def build(rows_per_core=ROWS_TOTAL // N_CORES):
    nc = bacc.Bacc(
        "TRN2", target_bir_lowering=False, debug=False, num_devices=N_CORES
    )
    x_ext = nc.declare_dram_parameter("x", [rows_per_core, K], f32, isOutput=False)
    wt_ext = nc.declare_dram_parameter("wt", [K, N], f32, isOutput=False)
    out_ext = nc.declare_dram_parameter(
        "out", [rows_per_core, N], f16, isOutput=True
    )

    MT = rows_per_core // 128

    with tile.TileContext(nc) as tc:
        with (
            tc.tile_pool(name="xin", bufs=3) as xin,  # [128,K] f32 x loads
            tc.tile_pool(name="wld", bufs=WCACHE) as wld,  # cached raw W
            tc.tile_pool(name="wtmp", bufs=4) as wtmp,  # W 11..15 bounce
            tc.tile_pool(name="scaled", bufs=1) as scaled,  # x*si+MAGIC f32
            tc.tile_pool(name="qtmp", bufs=1) as qtmp,  # qx bf16
            tc.tile_pool(name="qxt", bufs=3) as qxtp,  # [128,KT,128] bf16 x^T
            tc.tile_pool(name="outp", bufs=1) as outp,  # [128,N] f16 results
            tc.tile_pool(name="singles", bufs=1) as singles,
            tc.tile_pool(name="small", bufs=8) as small,  # [128,1] stats
            tc.tile_pool(name="pacc", bufs=8, space="PSUM") as pacc,
        ):
            negmagic_b = singles.tile([128, 1], f32)
            nc.vector.memset(negmagic_b, -MAGIC)
            wsums = singles.tile([128, KT], f32)
            qwT = singles.tile([128, KT, N], fp8)

            w_tiles = {}

            def w_load(kt, pool, tag):
                wt_t = pool.tile([128, K], f32, tag=tag, name=f"w{kt}")
                nc.sync.dma_start(out=wt_t, in_=wt_ext[ts(kt, 128), :])
                w_tiles[kt] = wt_t

            x_tiles = {}

            def x_load(mi):
                x_t = xin.tile([128, K], f32, tag="xin", name=f"x{mi}")
                nc.sync.dma_start(out=x_t, in_=x_ext[ts(mi, 128), :])
                x_tiles[mi] = x_t

            # ---- sync ring: uncached W tiles first, x0 early, then the
            # cached W tiles
            w_load(WCACHE, wtmp, "wtmp")
            w_load(WCACHE + 1, wtmp, "wtmp")
            x_load(0)
            for kt in range(WCACHE + 2, KT):
                w_load(kt, wtmp, "wtmp")
            for kt in range(WCACHE):
                w_load(kt, wld, "wld")

            # ---- x-quant pieces
            amcs = {}

            def xq_dve(mi):
                x_t = x_tiles[mi]
                amax = small.tile([128, 1], f32, tag="small")
                nc.vector.tensor_reduce(
                    out=amax, in_=x_t, axis=AxX, op=Alu.max,
                    apply_absolute_value=True,
                )
                amc = small.tile([128, 1], f32, tag="amc", name=f"amc{mi}")
                nc.vector.tensor_scalar_max(out=amc, in0=amax, scalar1=1e-5)
                rec = small.tile([128, 1], f32, tag="small")
                nc.vector.reciprocal(out=rec, in_=amc)
                si = small.tile([128, 1], f32, tag="small")
                nc.vector.tensor_scalar_mul(out=si, in0=rec, scalar1=127.0)
                xs = scaled.tile([128, K], f32, tag="scaled")
                nc.vector.tensor_scalar(
                    out=xs, in0=x_t, scalar1=si, scalar2=MAGIC,
                    op0=Alu.mult, op1=Alu.add,
                )
                amcs[mi] = amc
                return xs

            qxs = {}

            def xq_act(mi, xs):
                qx = qtmp.tile([128, K], bf16, tag="qtmp")
                nc.scalar.activation(out=qx, in_=xs, func=Act.Copy, bias=-MAGIC)
                qxs[mi] = qx

            qxTs = {}

            def xq_transpose(mi):
                qxT = qxtp.tile([128, KT, 128], bf16, tag="qxt", name=f"qxT{mi}")
                nc.sync.dma_start_transpose(out=qxT, in_=qxs.pop(mi))
                qxTs[mi] = qxT

            # ---- mean|W| during the stream: bounce tiles first (their
            # reduces must retire early to keep the bounce rotation off the
            # load critical path), x0's quant chain next (it fills the DVE
            # gap before the cached tiles arrive), then the cached tiles
            def ws_reduce(kt):
                nc.vector.tensor_reduce(
                    out=wsums[:, kt : kt + 1], in_=w_tiles[kt], axis=AxX,
                    op=Alu.add, apply_absolute_value=True,
                )

            for kt in range(WCACHE, KT):
                ws_reduce(kt)
            xq_act(0, xq_dve(0))
            for kt in range(WCACHE):
                ws_reduce(kt)
            wtot = small.tile([128, 1], f32, tag="small")
            nc.vector.tensor_reduce(out=wtot, in_=wsums, axis=AxX, op=Alu.add)
            # GPSIMD partition all-reduce replicates the grand total across
            # all 128 partitions without the PE ones-matmul round-trip (the
            # PSUM->DVE visibility latency alone measured ~2.2us)
            ptot_b = small.tile([128, 1], f32, tag="ptot")
            nc.gpsimd.partition_all_reduce(
                out_ap=ptot_b, in_ap=wtot, channels=128,
                reduce_op=bass_isa.ReduceOp.add,
            )
            meanc_b = small.tile([128, 1], f32, tag="s1")
            nc.vector.tensor_scalar(
                out=meanc_b, in0=ptot_b, scalar1=1.0 / (K * N), scalar2=1e-5,
                op0=Alu.mult, op1=Alu.max,
            )
            sw_b = singles.tile([128, 1], f32)
            nc.vector.reciprocal(out=sw_b, in_=meanc_b)
            q_b = singles.tile([128, 1], f32)
            nc.vector.tensor_scalar_mul(out=q_b, in0=meanc_b, scalar1=1.0 / 127.0)

            # ---- W pass2: u = w*sw + MAGIC (DVE in-place), then
            # Sign(u - MAGIC) -> fp8 (ACT); for integer n, sign(n) ==
            # clip(n, -1, 1)
            def wq(kt):
                wt_t = w_tiles[kt]
                nc.vector.tensor_scalar(
                    out=wt_t, in0=wt_t, scalar1=sw_b, scalar2=MAGIC,
                    op0=Alu.mult, op1=Alu.add,
                )
                nc.scalar.activation(
                    out=qwT[:, kt, :], in_=wt_t, func=Act.Sign, bias=negmagic_b
                )

            # sync ring after the W stream: x0's transpose (gates the first
            # matmul), x1 + transpose, the W11..15 re-reads, then x2 etc.
            xq_transpose(0)
            x_load(1)
            wq(0)
            wq(1)
            xq_act(1, xq_dve(1))
            xq_transpose(1)
            for kt in range(WCACHE, KT):
                w_load(kt, wtmp, "wtmp")  # re-read for ternarization
            for kt in range(2, KT):
                wq(kt)

            # ---- matmuls
            def mm(acc, qxT, kt, nq):
                nc.tensor.matmul(
                    acc, lhsT=qxT[:, kt, :], rhs=qwT[:, kt, ts(nq, 512)],
                    start=(kt == 0), stop=(kt == KT - 1),
                    skip_group_check=True,
                )

            def mk_accs(mi):
                return [
                    pacc.tile([128, 512], f32, tag="acc", name=f"acc_{mi}_{i}")
                    for i in range(NQ)
                ]

            def finish(mi, accs, chunked=False):
                cs = small.tile([128, 1], f32, tag="small")
                nc.vector.tensor_mul(cs, amcs.pop(mi), q_b)  # (amax/127)*meanc
                o_t = outp.tile([128, N], f16, tag="outp", name=f"o{mi}")
                for nq in range(NQ):
                    nc.scalar.activation(
                        out=o_t[:, ts(nq, 512)], in_=accs[nq],
                        func=Act.Copy, scale=cs,
                    )
                    if chunked:
                        nc.scalar.dma_start(
                            out=out_ext[ts(mi, 128), ts(nq, 512)],
                            in_=o_t[:, ts(nq, 512)],
                        )
                if not chunked:
                    nc.scalar.dma_start(out=out_ext[ts(mi, 128), :], in_=o_t)

            # ramp: rows 0,1 interleaved per k-tile, row 1 lagging LAG tiles
            accs0 = mk_accs(0)
            accs1 = mk_accs(1)
            for step in range(KT + LAG):
                if step < KT:
                    for nq in range(NQ):
                        mm(accs0[nq], qxTs[0], step, nq)
                if step >= LAG:
                    for nq in range(NQ):
                        mm(accs1[nq], qxTs[1], step - LAG, nq)
            finish(0, accs0)
            finish(1, accs1)
            del qxTs[0], qxTs[1]

            # steady rows
            for mi in range(2, MT):
                x_load(mi)
                xq_act(mi, xq_dve(mi))
                xq_transpose(mi)
                qxT = qxTs.pop(mi)
                accs = mk_accs(mi)
                if mi == MT - 1:
                    # nq-inner: each output chunk completes as soon as its
                    # 16 accumulations are done (shorter kernel tail)
                    for nq in range(NQ):
                        for kt in range(KT):
                            mm(accs[nq], qxT, kt, nq)
                else:
                    for kt in range(KT):
                        for nq in range(NQ):
                            mm(accs[nq], qxT, kt, nq)
                finish(mi, accs, chunked=(mi == MT - 1))

    nc.compile()
    return nc


_NC_CACHE = {}


def _get_nc(rows_per_core):
    if rows_per_core not in _NC_CACHE:
        _NC_CACHE[rows_per_core] = build(rows_per_core)
    return _NC_CACHE[rows_per_core]


def run(x, weight, **spmd_kwargs):
    x = np.ascontiguousarray(np.asarray(x, dtype=np.float32))
    weight = np.asarray(weight, dtype=np.float32)
    b, s, k = x.shape
    rows = b * s
    rpc = rows // N_CORES
    xr = x.reshape(rows, k)
    wt = np.ascontiguousarray(weight.T)
    nc = _get_nc(rpc)
    in_maps = [
        {"x": xr[i * rpc : (i + 1) * rpc], "wt": wt} for i in range(N_CORES)
    ]
    res = run_bass_kernel_spmd(
        nc, in_maps, core_ids=list(range(N_CORES)), **spmd_kwargs
    )
    out = np.concatenate(
        [res.results[i]["out"] for i in range(N_CORES)], axis=0
    )
    return out.reshape(b, s, N), res


def kernel(x, weight):
    out, _ = run(x, weight)
    return out


# revision 24
# speedup vs baseline: 1.0628x; 1.0495x over previous
"""BitLinear (int8-activation x ternary-weight) matmul on 8 TRN2 NeuronCores.

Full inputs: x [4, 4096, 2048] f32, weight [2048, 2048] f32.
Output: [4, 4096, 2048] fp16 = ((qx @ qw.T) / si / sw).astype(f16).

Data-parallel over the 16384 rows (2048 rows/core). The weight is
replicated; each core computes mean|W| on-device during a single
streaming W read, ternarizes W into fp8 {-1,0,1}, and runs
bf16(lhsT=qx^T) x fp8(qw^T) matmuls with fp32 PSUM accumulation
(exact for these integer values). Dequant (acc * amax/127 * mean|W|)
is fused into the PSUM->SBUF fp16 copy on the ScalarEngine.

Head scheduling (what decides time-to-first-matmul, measured on HW):
- ALL input loads ride the sync HWDGE ring alone, in order
  W11..W15, x0, W0..W10. The five tiles that cannot stay cached in
  SBUF (11..15) are loaded FIRST through a 3-buf bounce pool: their
  |w|-reduces retire early, so no load ever waits on a DVE reduce of
  a previous bounce occupant (the 2-buf tail rotation cost v3 ~13us
  of serial DMA<->DVE interlock), and the re-reads after sw recycle
  the same bufs without Sign-gating stalls.
- DMA-xbar transposes are emitted after the W stream (the transpose
  serialization guard makes later HWDGE DMAs wait on them), and the
  x2 load follows the re-reads immediately so row 2 is never input-
  blocked when the ramp retires.
- Putting W loads on the scalar ring was measured WORSE (ACT-queue
  head-of-line + semaphore-lane aliasing with the transpose guard).
- Rows 0,1 interleave across arriving ternarized k-tiles with row 1
  lagging 4 tiles (its qxT dependency never head-of-line blocks row
  0's stream in the PE queue); rows 2..15 then run back-to-back at
  the PE rate (~216ns per [128x128]x[128x512] bf16 matmul).

Activation quantization uses the fp32 magic-number trick on DVE
(x*si + 1.5*2^23 rounds to nearest-even) with the un-bias into bf16
on ACT. fp8 DoubleRow was tried: it trips the board GPIO power
throttle (PE clamped to 13/16 clock), cancelling the gain.
"""

import numpy as np

import concourse.mybir as mybir
import concourse.tile as tile
from concourse import bacc
from concourse.bass import ts
from concourse.bass_utils import run_bass_kernel_spmd

N_CORES = 8
ROWS_TOTAL = 4 * 4096
K = 2048
N = 2048
MAGIC = 12582912.0  # 1.5*2^23: fp32 round-to-nearest-even (both signs)

KT = K // 128  # 16 k-tiles
NQ = N // 512  # 4 psum-width output chunks
WCACHE = 11  # raw W k-tiles 0..WCACHE-1 held in SBUF; the rest re-read
LAG = 4  # row-1 k-tile lag in the ramp

f32 = mybir.dt.float32
bf16 = mybir.dt.bfloat16
f16 = mybir.dt.float16
fp8 = mybir.dt.float8e4
Alu = mybir.AluOpType
Act = mybir.ActivationFunctionType
AxX = mybir.AxisListType.X


def build(rows_per_core=ROWS_TOTAL // N_CORES):
    nc = bacc.Bacc(
        "TRN2", target_bir_lowering=False, debug=False, num_devices=N_CORES
    )
    x_ext = nc.declare_dram_parameter("x", [rows_per_core, K], f32, isOutput=False)
    wt_ext = nc.declare_dram_parameter("wt", [K, N], f32, isOutput=False)
    out_ext = nc.declare_dram_parameter(
        "out", [rows_per_core, N], f16, isOutput=True
    )

    MT = rows_per_core // 128

    with tile.TileContext(nc) as tc:
        with (
            tc.tile_pool(name="xin", bufs=3) as xin,  # [128,K] f32 x loads
            tc.tile_pool(name="wld", bufs=WCACHE) as wld,  # cached raw W
            tc.tile_pool(name="wtmp", bufs=5) as wtmp,  # W 11..15 bounce
            tc.tile_pool(name="qtmp", bufs=1) as qtmp,  # qx bf16
            tc.tile_pool(name="qxt", bufs=3) as qxtp,  # [128,KT,128] bf16 x^T
            tc.tile_pool(name="outp", bufs=1) as outp,  # [128,N] f16 results
            tc.tile_pool(name="singles", bufs=1) as singles,
            tc.tile_pool(name="small", bufs=8) as small,  # [128,1] stats
            tc.tile_pool(name="pacc", bufs=8, space="PSUM") as pacc,
        ):
            ones_mat = singles.tile([128, 128], f32)
            nc.vector.memset(ones_mat, 1.0)
            negmagic_b = singles.tile([128, 1], f32)
            nc.vector.memset(negmagic_b, -MAGIC)
            wsums = singles.tile([128, KT], f32)
            qwT = singles.tile([128, KT, N], fp8)

            w_tiles = {}

            def w_load(kt, pool, tag):
                wt_t = pool.tile([128, K], f32, tag=tag, name=f"w{kt}")
                nc.sync.dma_start(out=wt_t, in_=wt_ext[ts(kt, 128), :])
                w_tiles[kt] = wt_t

            x_tiles = {}

            def x_load(mi):
                x_t = xin.tile([128, K], f32, tag="xin", name=f"x{mi}")
                nc.sync.dma_start(out=x_t, in_=x_ext[ts(mi, 128), :])
                x_tiles[mi] = x_t

            # ---- sync ring: uncached W tiles first, x0 early, then the
            # cached W tiles
            w_load(WCACHE, wtmp, "wtmp")
            w_load(WCACHE + 1, wtmp, "wtmp")
            x_load(0)
            for kt in range(WCACHE + 2, KT):
                w_load(kt, wtmp, "wtmp")
            for kt in range(WCACHE):
                w_load(kt, wld, "wld")

            # ---- x-quant pieces
            amcs = {}

            def xq_dve(mi):
                x_t = x_tiles[mi]
                amax = small.tile([128, 1], f32, tag="small")
                nc.vector.tensor_reduce(
                    out=amax, in_=x_t, axis=AxX, op=Alu.max,
                    apply_absolute_value=True,
                )
                amc = small.tile([128, 1], f32, tag="amc", name=f"amc{mi}")
                nc.vector.tensor_scalar_max(out=amc, in0=amax, scalar1=1e-5)
                rec = small.tile([128, 1], f32, tag="small")
                nc.vector.reciprocal(out=rec, in_=amc)
                si = small.tile([128, 1], f32, tag="small")
                nc.vector.tensor_scalar_mul(out=si, in0=rec, scalar1=127.0)
                # in-place: raw x is dead after the amax reduce above
                nc.vector.tensor_scalar(
                    out=x_t, in0=x_t, scalar1=si, scalar2=MAGIC,
                    op0=Alu.mult, op1=Alu.add,
                )
                amcs[mi] = amc
                return x_t

            qxs = {}

            def xq_act(mi, xs):
                qx = qtmp.tile([128, K], bf16, tag="qtmp")
                nc.scalar.activation(out=qx, in_=xs, func=Act.Copy, bias=-MAGIC)
                qxs[mi] = qx

            qxTs = {}

            def xq_transpose(mi):
                qxT = qxtp.tile([128, KT, 128], bf16, tag="qxt", name=f"qxT{mi}")
                nc.sync.dma_start_transpose(out=qxT, in_=qxs.pop(mi))
                qxTs[mi] = qxT

            # ---- mean|W| during the stream: bounce tiles first (their
            # reduces must retire early to keep the bounce rotation off the
            # load critical path), x0's quant chain next (it fills the DVE
            # gap before the cached tiles arrive), then the cached tiles
            def ws_reduce(kt):
                nc.vector.tensor_reduce(
                    out=wsums[:, kt : kt + 1], in_=w_tiles[kt], axis=AxX,
                    op=Alu.add, apply_absolute_value=True,
                )

            for kt in range(WCACHE, KT):
                ws_reduce(kt)
            xq_act(0, xq_dve(0))
            for kt in range(WCACHE):
                ws_reduce(kt)
            wtot = small.tile([128, 1], f32, tag="small")
            nc.vector.tensor_reduce(out=wtot, in_=wsums, axis=AxX, op=Alu.add)
            # ones_mat.T @ wtot replicates the grand total across all 128
            # partitions so the scale math stays [128,1]
            ptot_b = pacc.tile([128, 1], f32, tag="acc", name="ptot_b")
            nc.tensor.matmul(ptot_b, lhsT=ones_mat, rhs=wtot, start=True, stop=True)
            meanc_b = small.tile([128, 1], f32, tag="s1")
            nc.vector.tensor_scalar(
                out=meanc_b, in0=ptot_b, scalar1=1.0 / (K * N), scalar2=1e-5,
                op0=Alu.mult, op1=Alu.max,
            )
            sw_b = singles.tile([128, 1], f32)
            nc.vector.reciprocal(out=sw_b, in_=meanc_b)
            q_b = singles.tile([128, 1], f32)
            nc.vector.tensor_scalar_mul(out=q_b, in0=meanc_b, scalar1=1.0 / 127.0)

            # ---- W pass2: u = w*sw + MAGIC (DVE in-place), then
            # Sign(u - MAGIC) -> fp8 (ACT); for integer n, sign(n) ==
            # clip(n, -1, 1)
            def wq(kt):
                wt_t = w_tiles[kt]
                nc.vector.tensor_scalar(
                    out=wt_t, in0=wt_t, scalar1=sw_b, scalar2=MAGIC,
                    op0=Alu.mult, op1=Alu.add,
                )
                nc.scalar.activation(
                    out=qwT[:, kt, :], in_=wt_t, func=Act.Sign, bias=negmagic_b
                )

            # sync ring after the W stream: x0's transpose (gates the first
            # matmul), x1 + transpose, the W11..15 re-reads, then x2 etc.
            xq_transpose(0)
            x_load(1)
            wq(0)
            wq(1)
            xq_act(1, xq_dve(1))
            xq_transpose(1)
            for kt in range(WCACHE, KT):
                w_load(kt, wtmp, "wtmp")  # re-read for ternarization
            for kt in range(2, KT):
                wq(kt)

            # ---- matmuls
            def mm(acc, qxT, kt, nq):
                nc.tensor.matmul(
                    acc, lhsT=qxT[:, kt, :], rhs=qwT[:, kt, ts(nq, 512)],
                    start=(kt == 0), stop=(kt == KT - 1),
                    skip_group_check=True,
                )

            def mk_accs(mi):
                return [
                    pacc.tile([128, 512], f32, tag="acc", name=f"acc_{mi}_{i}")
                    for i in range(NQ)
                ]

            def finish(mi, accs, chunked=False):
                cs = small.tile([128, 1], f32, tag="small")
                nc.vector.tensor_mul(cs, amcs.pop(mi), q_b)  # (amax/127)*meanc
                o_t = outp.tile([128, N], f16, tag="outp", name=f"o{mi}")
                for nq in range(NQ):
                    nc.scalar.activation(
                        out=o_t[:, ts(nq, 512)], in_=accs[nq],
                        func=Act.Copy, scale=cs,
                    )
                    if chunked:
                        nc.scalar.dma_start(
                            out=out_ext[ts(mi, 128), ts(nq, 512)],
                            in_=o_t[:, ts(nq, 512)],
                        )
                if not chunked:
                    nc.scalar.dma_start(out=out_ext[ts(mi, 128), :], in_=o_t)

            # ramp: rows 0,1 interleaved per k-tile, row 1 lagging LAG tiles
            accs0 = mk_accs(0)
            accs1 = mk_accs(1)
            for step in range(KT + LAG):
                if step < KT:
                    for nq in range(NQ):
                        mm(accs0[nq], qxTs[0], step, nq)
                if step >= LAG:
                    for nq in range(NQ):
                        mm(accs1[nq], qxTs[1], step - LAG, nq)
            finish(0, accs0)
            finish(1, accs1)
            del qxTs[0], qxTs[1]

            # steady rows
            for mi in range(2, MT):
                x_load(mi)
                xq_act(mi, xq_dve(mi))
                xq_transpose(mi)
                qxT = qxTs.pop(mi)
                accs = mk_accs(mi)
                if mi == MT - 1:
                    # nq-inner: each output chunk completes as soon as its
                    # 16 accumulations are done (shorter kernel tail)
                    for nq in range(NQ):
                        for kt in range(KT):
                            mm(accs[nq], qxT, kt, nq)
                else:
                    for kt in range(KT):
                        for nq in range(NQ):
                            mm(accs[nq], qxT, kt, nq)
                finish(mi, accs, chunked=(mi == MT - 1))

    nc.compile()
    return nc


_NC_CACHE = {}


def _get_nc(rows_per_core):
    if rows_per_core not in _NC_CACHE:
        _NC_CACHE[rows_per_core] = build(rows_per_core)
    return _NC_CACHE[rows_per_core]


def run(x, weight, **spmd_kwargs):
    x = np.ascontiguousarray(np.asarray(x, dtype=np.float32))
    weight = np.asarray(weight, dtype=np.float32)
    b, s, k = x.shape
    rows = b * s
    rpc = rows // N_CORES
    xr = x.reshape(rows, k)
    wt = np.ascontiguousarray(weight.T)
    nc = _get_nc(rpc)
    in_maps = [
        {"x": xr[i * rpc : (i + 1) * rpc], "wt": wt} for i in range(N_CORES)
    ]
    res = run_bass_kernel_spmd(
        nc, in_maps, core_ids=list(range(N_CORES)), **spmd_kwargs
    )
    out = np.concatenate(
        [res.results[i]["out"] for i in range(N_CORES)], axis=0
    )
    return out.reshape(b, s, N), res


def kernel(x, weight):
    out, _ = run(x, weight)
    return out
